# revision 1
# baseline (speedup 1.0000x reference)
"""Multi-head attention (B=2, S=2048, D=1024, H=16, causal + key-pad mask)
as an 8-core Trainium2 Bass/Tile SPMD kernel.

Sharding: data parallel over the 2 batches (4 cores each); within a batch
group, tensor parallel over heads (4 heads/core) for the QKV projections and
attention, then a per-q-tile AllGather of the (unnormalized) head outputs +
softmax denominators, and a rank-dynamic row-sliced O-projection (each core
produces 512 output rows). Matmuls run as float32r (fp32 storage, ~1 cyc/row
PE throughput). Softmax skips max-subtraction (scores are O(5) for this
problem class), applies the key-pad mask through the exp bias (per-partition
in the scores-transposed [k, q] layout) and the causal mask via an
identity-matmul accumulation of a triangular tile on the PE.

self-contained: includes a workaround for the walrus per-instruction
sync-wait limit and an NTFF-profile hook shim.
"""
import sys
import types

import numpy as np

import bass_rust
import concourse.bass as bass
import concourse.mybir as mybir
import concourse.tile as tile


# ---- walrus sync-wait limit workaround ----------------------------------
# This walrus build rejects instructions carrying more than one sem wait
# ("Too many sync wait commands"). Tile emits multi-wait instructions (the
# final drain, matmuls waiting on several DMA queues). Split excess waits
# onto same-engine NoOps placed immediately before the instruction --
# serial waits on one sequencer are semantically identical.
_WSPLIT_COUNTER = [0]


def _split_excess_waits(nc, limit=1):
    for fn in nc.m.functions:
        for bb in fn.blocks:
            out = []
            changed = False
            for inst in bb.instructions:
                si = inst.sync_info
                waits = list(si.on_wait) if si is not None and si.on_wait else []
                if len(waits) > limit:
                    extra, keep = waits[:-limit], waits[-limit:]
                    for s in range(0, len(extra), limit):
                        _WSPLIT_COUNTER[0] += 1
                        nop = mybir.InstNoOp(
                            name=f"I-wsplit-{_WSPLIT_COUNTER[0]}", ins=[], outs=[]
                        )
                        nop.engine = inst.engine
                        nop.sync_info = bass_rust.SyncInfo(
                            on_wait=extra[s : s + limit], on_update=[]
                        )
                        out.append(nop)
                    si.on_wait = keep
                    changed = True
                out.append(inst)
            if changed:
                bb.instructions = out


def _install_tile_patch():
    if getattr(tile.TileContext, "_wait_split_patched", False):
        return
    orig_exit = tile.TileContext.__exit__

    def __exit__(self, exc_type, exc_val, exc_tb):
        r = orig_exit(self, exc_type, exc_val, exc_tb)
        if exc_type is None:
            _split_excess_waits(self.nc)
        return r

    tile.TileContext.__exit__ = __exit__
    tile.TileContext._wait_split_patched = True


_install_tile_patch()


# ---- NTFF profile hook shim (axon deployments missing antenv.axon_hooks) --
def _install_ntff_hook():
    try:
        import antenv.axon_hooks  # noqa: F401
        return
    except ImportError:
        pass
    try:
        from trn_agent_boot.trn_boot import _ntff_profile_via_ctypes

        hook = _ntff_profile_via_ctypes("/opt/axon/libaxon_pjrt.so")
    except Exception:
        hook = None
    m = types.ModuleType("antenv.axon_hooks")
    m.get_axon_ntff_profile_hook = lambda: hook
    m.set_axon_ntff_profile_hook = lambda h: None
    sys.modules["antenv.axon_hooks"] = m


_install_ntff_hook()

from concourse.bass_utils import run_bass_kernel_spmd  # noqa: E402

f32 = mybir.dt.float32
f32r = mybir.dt.float32r

B, S, D, H, HD = 2, 2048, 1024, 16, 64
HPC, GROUP = 4, 4          # heads per core, cores per batch
HC = HPC * HD              # 256 projection cols per core
NKT = S // 128             # 16 k-tiles
NQT = S // 512             # 4 q-tiles
QT = 512                   # q-tile width
SCALE = 1.0 / np.sqrt(HD)  # 0.125
NEG = -1.0e9
KCH = D // 128             # 8 contraction chunks

REPLICA_GROUPS = [[0, 1, 2, 3], [4, 5, 6, 7]]


def r32(ap):
    return ap.bitcast(f32r)


def build():
    nc = bass.Bass()
    dp = nc.declare_dram_parameter
    xT = dp("xT", [D, S], f32r, isOutput=False)
    wqT = dp("wqT", [D, HC], f32r, isOutput=False)
    wkT = dp("wkT", [D, HC], f32r, isOutput=False)
    wvT = dp("wvT", [D, HC], f32r, isOutput=False)
    woT = dp("woT", [D, D], f32r, isOutput=False)
    bq = dp("bq", [128, 2], f32, isOutput=False)
    bk = dp("bk", [128, 2], f32, isOutput=False)
    bv = dp("bv", [1, HC], f32r, isOutput=False)
    bo = dp("bo", [1, D], f32r, isOutput=False)
    padb = dp("padb", [128, NKT], f32, isOutput=False)
    tri = dp("tri", [128, 128], f32r, isOutput=False)
    iden = dp("iden", [128, 128], f32r, isOutput=False)
    esel = dp("esel", [H, D], f32r, isOutput=False)
    out = dp("out", [QT, D], f32, isOutput=True)

    with tile.TileContext(nc) as tc:
        _body(nc, tc, locals())
    return nc


def _body(nc, tc, t):
    xT, wqT, wkT, wvT, woT = t["xT"], t["wqT"], t["wkT"], t["wvT"], t["woT"]
    bq, bk, bv, bo, padb, tri = t["bq"], t["bk"], t["bv"], t["bo"], t["padb"], t["tri"]
    iden = t["iden"]
    esel = t["esel"]
    out = t["out"]

    ctx_pools = []

    def pool(name, bufs, space="SBUF"):
        p = tc.tile_pool(name=name, bufs=bufs, space=space)
        ctx_pools.append(p)
        return p.__enter__()

    dram_pool = pool("dram", 1, space="DRAM")
    HCA = HPC * (HD + 1)  # 260 rows: 4 heads x (64 d' + denom)
    ag_in = dram_pool.tile([NQT, HCA, QT], f32r)
    ag_out = dram_pool.tile([NQT, GROUP * HCA, QT], f32r)

    const = pool("const", 1)
    probs_pool = pool("probs", 4)
    stage_pool = pool("stage", 3)
    oproj_pool = pool("oproj", 1)
    wo_pool = pool("wo", 1)
    outsb_pool = pool("outsb", 2)

    ps_proj = pool("ps_proj", 2, space="PSUM")
    ps_st = pool("ps_st", 2, space="PSUM")
    ps_ot = pool("ps_ot", 2, space="PSUM")
    qk_ctx = tc.tile_pool(name="qk", bufs=1)
    qk_pool = qk_ctx.__enter__()
    vh_ctx = tc.tile_pool(name="vh", bufs=1)
    vh_pool = vh_ctx.__enter__()

    # ---- resident inputs -------------------------------------------------
    xt = const.tile([128, KCH, S], f32r)      # xT, chunk-major
    wq_t = const.tile([128, KCH, HC], f32r)
    wk_t = const.tile([128, KCH, HC], f32r)
    wv_t = const.tile([128, KCH, HC], f32r)
    bq_t = const.tile([128, 2], f32)
    nc.sync.dma_start(bq_t[:], bq[:])
    bk_t = const.tile([128, 2], f32)
    nc.sync.dma_start(bk_t[:], bk[:])
    padb_t = const.tile([128, NKT], f32)
    nc.sync.dma_start(padb_t[:], padb[:])
    tri_t = const.tile([128, 128], f32r)
    nc.sync.dma_start(tri_t[:], tri[:])
    iden_t = const.tile([128, 128], f32r)
    nc.sync.dma_start(iden_t[:], iden[:])
    bo_t = const.tile([1, D], f32r)
    nc.sync.dma_start(bo_t[:], bo[:])
    bv_row = const.tile([1, HC], f32r)
    nc.sync.dma_start(bv_row[:], bv[:])

    xT_r = xT.rearrange("(c p) s -> p c s", p=128)
    wqT_r = wqT.rearrange("(c p) j -> p c j", p=128)
    wkT_r = wkT.rearrange("(c p) j -> p c j", p=128)
    wvT_r = wvT.rearrange("(c p) j -> p c j", p=128)
    for k in range(KCH):
        nc.sync.dma_start(wq_t[:, k], wqT_r[:, k])
        nc.sync.dma_start(wk_t[:, k], wkT_r[:, k])
        nc.sync.dma_start(wv_t[:, k], wvT_r[:, k])
        nc.sync.dma_start(xt[:, k], xT_r[:, k])

    ones_f = const.tile([128, 64], f32)
    nc.any.memset(ones_f[:], 1.0)
    ones1 = const.tile([1, 128], f32r)
    nc.vector.tensor_copy(ones1[0:1, 0:64], ones_f[0:1, :])
    nc.vector.tensor_copy(ones1[0:1, 64:128], ones_f[0:1, :])
    bvb = const.tile([128, HC], f32)
    bv_ps = ps_proj.tile([128, HC], f32, tag="proj")
    nc.tensor.matmul(bv_ps[:], ones1[:], bv_row[:], start=True, stop=True)
    nc.vector.tensor_copy(bvb[:], bv_ps[:])

    # projection outputs
    qh_t = qk_pool.tile([128, 2, S], f32r)    # qhT: [j-in-tile, j-tile, s]
    kh_t = qk_pool.tile([128, 2, S], f32r)
    vh_t = vh_pool.tile([128, NKT, HPC, HD + 1], f32r)  # [s-in-tile, s-tile, head, d'+ones]
    nc.vector.tensor_copy(
        vh_t[:, :, :, HD : HD + 1].rearrange("p t h o -> p (t h) o"),
        ones_f[:, 0 : NKT * HPC].rearrange("p (f o) -> p f o", o=1),
    )

    def proj_qk(w_t, b_t, out_t, jt, c):
        """one [128, 512] tile of qhT/khT: out partition=j, free=s."""
        ps = ps_proj.tile([128, QT], f32, tag="proj")
        for k in range(KCH):
            nc.tensor.matmul(
                ps[:],
                r32(w_t[:, k, jt * 128 : (jt + 1) * 128]),
                r32(xt[:, k, c * QT : (c + 1) * QT]),
                start=(k == 0),
                stop=(k == KCH - 1),
            )
            if k % 2 == 1:
                yield
        nc.vector.tensor_scalar_add(
            out_t[:, jt, c * QT : (c + 1) * QT], ps[:], b_t[:, jt : jt + 1]
        )

    def proj_v(st_):
        """one s-tile of vh: out partition=s, free=[4 heads x 64]."""
        ps = ps_proj.tile([128, HC], f32, tag="proj")
        for k in range(KCH):
            nc.tensor.matmul(
                ps[:],
                r32(xt[:, k, st_ * 128 : (st_ + 1) * 128]),
                r32(wv_t[:, k, :]),
                start=(k == 0),
                stop=(k == KCH - 1),
            )
            if k % 2 == 1:
                yield
        nc.vector.tensor_tensor(
            vh_t[:, st_, :, 0:HD],
            ps[:].rearrange("p (h d) -> p h d", h=HPC),
            bvb[:].rearrange("p (h d) -> p h d", h=HPC),
            mybir.AluOpType.add,
        )

    def attention_qtile(qi, filler=None):
        q0 = qi * QT
        nk = 4 * (qi + 1)
        for pair in range(2):  # heads (2p, 2p+1) at partitions 0-63 / 64-127 of tile jt=pair
            ot0 = ps_ot.tile([HD + 1, QT], f32, tag="ot")
            ot1 = ps_ot.tile([HD + 1, QT], f32, tag="ot")
            ots = (ot0, ot1)
            for kt in range(nk):
                if filler is not None:
                    filler()
                k0 = kt * 128
                d0 = max(0, k0 - q0)  # first valid q-col in this tile
                st = ps_st.tile([128, 2, QT], f32, tag="st")
                diag = k0 >= q0
                for hh in range(2):
                    nc.tensor.matmul(
                        st[:, hh, d0:QT],
                        r32(kh_t[hh * 64 : hh * 64 + 64, pair, k0 : k0 + 128]),
                        r32(qh_t[hh * 64 : hh * 64 + 64, pair, q0 + d0 : q0 + QT]),
                        start=True,
                        stop=True,
                    )
                if diag:  # causal mask on the [128,128] diagonal block (DVE)
                    nc.vector.tensor_tensor(
                        st[:, :, d0 : d0 + 128],
                        st[:, :, d0 : d0 + 128],
                        tri_t[:].bitcast(f32).rearrange(
                            "p (o n) -> p o n", o=1
                        ).broadcast_to([128, 2, 128]),
                        mybir.AluOpType.add,
                    )
                probs = probs_pool.tile([128, 2, QT], f32r, tag="probs")
                nc.scalar.activation(
                    probs[:, :, d0:QT],
                    st[:, :, d0:QT],
                    mybir.ActivationFunctionType.Exp,
                    bias=padb_t[:, kt : kt + 1],
                    scale=float(SCALE),
                )
                for hh in range(2):
                    h = 2 * pair + hh
                    nc.tensor.matmul(
                        ots[hh][:, d0:QT],
                        r32(vh_t[:, kt, h, :]),
                        r32(probs[:, hh, d0:QT]),
                        start=(kt == 0),
                        stop=(kt == nk - 1),
                    )
            # stage unnormalized oT + denominator row for the AllGather
            for hh in range(2):
                if filler is not None:
                    filler()
                h = 2 * pair + hh
                stg = stage_pool.tile([HD + 1, QT], f32r, tag="stg")
                nc.vector.tensor_copy(stg[:], ots[hh][:])
                nc.sync.dma_start(
                    ag_in[qi, h * (HD + 1) : (h + 1) * (HD + 1), :], stg[:]
                )

    # ---- emission: projections finely interleaved with attention ---------
    def proj_units(c):
        units = []
        for jt in range(2):
            units.append(lambda jt=jt, c=c: proj_qk(wk_t, bk_t, kh_t, jt, c))
            units.append(lambda jt=jt, c=c: proj_qk(wq_t, bq_t, qh_t, jt, c))
        for st_ in range(4 * c, 4 * c + 4):
            units.append(lambda st_=st_: proj_v(st_))
        return units

    class Filler:
        def __init__(self, units, budget):
            self.units = list(units)
            self.gen = None
            self.budget = budget

        def __call__(self):
            for _ in range(self.budget):
                if self.gen is None:
                    if not self.units:
                        return
                    self.gen = self.units.pop(0)()
                try:
                    next(self.gen)
                except StopIteration:
                    self.gen = None

        def flush(self):
            while self.units or self.gen is not None:
                if self.gen is None:
                    self.gen = self.units.pop(0)()
                for _ in self.gen:
                    pass
                self.gen = None

    Filler(proj_units(0), 1).flush()
    wo_sbs = []
    for c in range(NQT):
        pending = proj_units(c + 1) if c + 1 < NQT else []
        n_att = 2 * 4 * (c + 1) + 4
        total_steps = len(pending) * 5
        budget = max(1, (total_steps + n_att - 1) // n_att)
        filler = Filler(pending, budget)
        attention_qtile(c, filler)
        filler.flush()
        # AllGather this q-tile's oT across the 4-core group
        nc.gpsimd.collective_compute(
            "AllGather",
            mybir.AluOpType.bypass,
            replica_groups=REPLICA_GROUPS,
            ins=[ag_in[c]],
            outs=[ag_out[c]],
        )
        if c == 1:
            # prefetch Wo half 0 during remaining attention
            wo_sb = wo_pool.tile([128, KCH, QT], f32r, tag="wo")
            nc.sync.dma_start(
                wo_sb[:],
                woT[:, 0:QT].rearrange("(c p) j -> p c j", p=128),
            )
            wo_sbs.append(wo_sb)

    # attention done: free qhT/khT/vh SBUF for the O-projection working set
    vh_ctx.__exit__(None, None, None)
    qk_ctx.__exit__(None, None, None)
    # prefetch Wo half 1 into freed space (overlaps the last AllGather)
    wo_ctx_b = tc.tile_pool(name="wo_b", bufs=1)
    wo_pool_b = wo_ctx_b.__enter__()
    ctx_pools.append(wo_ctx_b)
    wo_sb1 = wo_pool_b.tile([128, KCH, QT], f32r, tag="wo_b")
    nc.sync.dma_start(
        wo_sb1[:], woT[:, QT : 2 * QT].rearrange("(c p) j -> p c j", p=128)
    )
    wo_sbs.append(wo_sb1)

    # ---- O-projection on own 512-row slice (rank-dynamic chunk) ----------
    rank = nc.gpsimd.partition_id() % GROUP
    ag = oproj_pool.tile([128, KCH, QT], f32r)
    den = oproj_pool.tile([H, QT], f32)
    # per-head oT rows (skip denom rows), packed 2 heads per 128-partition chunk;
    # head h = 2c + a lives at ag partitions a*64..a*64+63, chunk c
    # denominator rows first: the reciprocal overlaps the big head load
    src_den = ag_out[:].rearrange("q (h r) s -> q h r s", r=HD + 1)
    nc.gpsimd.dma_start(
        den[:],
        src_den[bass.ds(rank, 1), :, HD, :].rearrange("o h s -> (o h) s"),
    )
    src_heads = ag_out[:].rearrange("q (c a r) s -> q c a r s", a=2, r=HD + 1)
    for a in range(2):
        nc.gpsimd.dma_start(
            ag[a * HD : (a + 1) * HD, :, :],
            src_heads[bass.ds(rank, 1), :, a, 0:HD, :].rearrange(
                "o c p s -> (o p) c s"
            ),
        )
    den_r = oproj_pool.tile([H, QT], f32r)
    with nc.allow_low_precision(reason="fp32r mantissa rounding"):
        nc.vector.reciprocal(den_r[:], den[:])
    for dc in range(KCH):
        esel_c = oproj_pool.tile([H, 128], f32r, tag="esel", bufs=2)
        nc.sync.dma_start(esel_c[:], esel[:, dc * 128 : (dc + 1) * 128])
        bcst = ps_proj.tile([128, QT], f32, tag="proj")
        nc.tensor.matmul(
            bcst[:], esel_c[:], den_r[:], start=True, stop=True
        )
        with nc.allow_low_precision(reason="fp32r mantissa rounding"):
            nc.vector.tensor_tensor(
                ag[:, dc, :], ag[:, dc, :], bcst[:], mybir.AluOpType.mult
            )
    for jh in range(2):
        wo_sb = wo_sbs[jh]
        for st_ in range(4):
            ps = ps_proj.tile([128, QT], f32, tag="proj")
            for dc in range(KCH):
                nc.tensor.matmul(
                    ps[:],
                    r32(ag[:, dc, st_ * 128 : (st_ + 1) * 128]),
                    r32(wo_sb[:, dc, :]),
                    start=(dc == 0),
                    stop=False,
                )
            nc.tensor.matmul(
                ps[:],
                r32(ones1[:]),
                r32(bo_t[:, jh * QT : (jh + 1) * QT]),
                start=False,
                stop=True,
            )
            osb = outsb_pool.tile([128, QT], f32, tag="osb")
            nc.vector.tensor_copy(osb[:], ps[:])
            nc.sync.dma_start(
                out[st_ * 128 : (st_ + 1) * 128, jh * QT : (jh + 1) * QT], osb[:]
            )

    for p in reversed(ctx_pools):
        p.__exit__(None, None, None)


# ---- host-side marshalling ----------------------------------------------


def make_inputs(q, pad_mask, Wq, bq, Wk, bk, Wv, bv, Wo, bo):
    """Build the 8 per-core input maps from full inputs."""
    tri_m = np.where(
        np.arange(128)[:, None] > np.arange(128)[None, :], np.float32(NEG), np.float32(0)
    ).astype(np.float32)
    esel_m = (np.arange(D)[None, :] // HD == np.arange(H)[:, None]).astype(np.float32)
    in_maps = []
    xTs = [np.ascontiguousarray(q[b].T).astype(np.float32) for b in range(B)]
    padbs = [
        np.ascontiguousarray(
            np.where(pad_mask[b], np.float32(NEG), np.float32(0))
            .astype(np.float32)
            .reshape(NKT, 128)
            .T
        )
        for b in range(B)
    ]
    for core in range(8):
        b, r = divmod(core, GROUP)
        sl = slice(r * HC, (r + 1) * HC)
        in_maps.append(
            {
                "xT": xTs[b],
                "wqT": np.ascontiguousarray(Wq[sl, :].T),
                "wkT": np.ascontiguousarray(Wk[sl, :].T),
                "wvT": np.ascontiguousarray(Wv[sl, :].T),
                "woT": np.ascontiguousarray(Wo.T),
                "bq": np.ascontiguousarray(bq[sl].reshape(2, 128).T),
                "bk": np.ascontiguousarray(bk[sl].reshape(2, 128).T),
                "bv": np.ascontiguousarray(bv[sl].reshape(1, HC)),
                "bo": np.ascontiguousarray(bo.reshape(1, D)),
                "padb": padbs[b],
                "tri": tri_m,
                "iden": np.eye(128, dtype=np.float32),
                "esel": esel_m,
            }
        )
    return in_maps


def assemble_output(results):
    full = np.empty((B, S, D), dtype=np.float32)
    for core in range(8):
        b, r = divmod(core, GROUP)
        full[b, r * QT : (r + 1) * QT, :] = results[core]["out"]
    return full


_NC_CACHE = [None]


def kernel(**inputs):
    """Full-input MHA forward. inputs: q, pad_mask, Wq, bq, Wk, bk, Wv, bv,
    Wo, bo (as produced by setup_inputs). Returns [B, S, D] float32."""
    if _NC_CACHE[0] is None:
        _NC_CACHE[0] = build()
    nc = _NC_CACHE[0]
    inputs = {k: np.asarray(v) for k, v in inputs.items()}
    in_maps = make_inputs(**inputs)
    res = run_bass_kernel_spmd(nc, in_maps, list(range(8)))
    return assemble_output(res.results)



# revision 5
# speedup vs baseline: 1.0977x; 1.0977x over previous
"""Multi-head attention (B=2, S=2048, D=1024, H=16, causal + key-pad mask)
as an 8-core Trainium2 Bass/Tile SPMD kernel.

Sharding: data parallel over the 2 batches (4 cores each); within a batch
group, tensor parallel over heads (4 heads/core) for the QKV projections and
attention. Head outputs are normalized (softmax denominator folded in) and
staged in bf16, AllGathered per q-tile across the 4-core group, and each core
then computes a 256-column slice of the O-projection for every q-tile as soon
as that tile's AllGather lands (overlapped with later attention tiles). The
full output is assembled column-wise on the host.

All matmul operands are bf16 (FWL weight loads, 1 cyc/row at any free size);
accumulation stays fp32 in PSUM. Softmax skips max-subtraction (scores are
O(5) for this problem class), applies the key-pad mask through the exp bias
(per-partition in the scores-transposed [k, q] layout) and the causal mask via
a DVE add of a triangular tile on the diagonal blocks.

self-contained: includes a workaround for the walrus per-instruction
sync-wait limit and an NTFF-profile hook shim.
"""
import sys
import types

import numpy as np

import bass_rust
import concourse.bass as bass
import concourse.mybir as mybir
import concourse.tile as tile


# ---- walrus sync-wait limit workaround ----------------------------------
# This walrus build rejects instructions carrying more than one sem wait
# ("Too many sync wait commands"). Tile emits multi-wait instructions (the
# final drain, matmuls waiting on several DMA queues). Split excess waits
# onto same-engine NoOps placed immediately before the instruction --
# serial waits on one sequencer are semantically identical.
_WSPLIT_COUNTER = [0]


def _split_excess_waits(nc, limit=1):
    for fn in nc.m.functions:
        for bb in fn.blocks:
            out = []
            changed = False
            for inst in bb.instructions:
                si = inst.sync_info
                waits = list(si.on_wait) if si is not None and si.on_wait else []
                if len(waits) > limit:
                    extra, keep = waits[:-limit], waits[-limit:]
                    for s in range(0, len(extra), limit):
                        _WSPLIT_COUNTER[0] += 1
                        nop = mybir.InstNoOp(
                            name=f"I-wsplit-{_WSPLIT_COUNTER[0]}", ins=[], outs=[]
                        )
                        nop.engine = inst.engine
                        nop.sync_info = bass_rust.SyncInfo(
                            on_wait=extra[s : s + limit], on_update=[]
                        )
                        out.append(nop)
                    si.on_wait = keep
                    changed = True
                out.append(inst)
            if changed:
                bb.instructions = out


def _install_tile_patch():
    if getattr(tile.TileContext, "_wait_split_patched", False):
        return
    orig_exit = tile.TileContext.__exit__

    def __exit__(self, exc_type, exc_val, exc_tb):
        r = orig_exit(self, exc_type, exc_val, exc_tb)
        if exc_type is None:
            _split_excess_waits(self.nc)
        return r

    tile.TileContext.__exit__ = __exit__
    tile.TileContext._wait_split_patched = True


_install_tile_patch()


# ---- NTFF profile hook shim (axon deployments missing antenv.axon_hooks) --
def _install_ntff_hook():
    try:
        import antenv.axon_hooks  # noqa: F401
        return
    except ImportError:
        pass
    try:
        from trn_agent_boot.trn_boot import _ntff_profile_via_ctypes

        hook = _ntff_profile_via_ctypes("/opt/axon/libaxon_pjrt.so")
    except Exception:
        hook = None
    m = types.ModuleType("antenv.axon_hooks")
    m.get_axon_ntff_profile_hook = lambda: hook
    m.set_axon_ntff_profile_hook = lambda h: None
    sys.modules["antenv.axon_hooks"] = m


_install_ntff_hook()

from concourse.bass_utils import run_bass_kernel_spmd  # noqa: E402

f32 = mybir.dt.float32
bf16 = mybir.dt.bfloat16

B, S, D, H, HD = 2, 2048, 1024, 16, 64
HPC, GROUP = 4, 4          # heads per core, cores per batch
HC = HPC * HD              # 256 projection cols per core
NKT = S // 128             # 16 k-tiles
NQT = S // 512             # 4 q-tiles
QT = 512                   # q-tile width
SCALE = 1.0 / np.sqrt(HD)  # 0.125
NEG = -1.0e9
KCH = D // 128             # 8 contraction chunks

REPLICA_GROUPS = [[0, 1, 2, 3], [4, 5, 6, 7]]


def build():
    nc = bass.Bass()
    dp = nc.declare_dram_parameter
    xT = dp("xT", [D, S], bf16, isOutput=False)
    wqT = dp("wqT", [D, HC], bf16, isOutput=False)
    wkT = dp("wkT", [D, HC], bf16, isOutput=False)
    wvT = dp("wvT", [D, HC], bf16, isOutput=False)
    woT = dp("woT", [D, HC], bf16, isOutput=False)
    bq = dp("bq", [128, 2], f32, isOutput=False)
    bk = dp("bk", [128, 2], f32, isOutput=False)
    bv = dp("bv", [1, HC], bf16, isOutput=False)
    bo = dp("bo", [128, 2], f32, isOutput=False)
    padb = dp("padb", [128, NKT], f32, isOutput=False)
    tri = dp("tri", [128, 128], f32, isOutput=False)
    out = dp("out", [NQT, 2, 128, QT], f32, isOutput=True)

    with tile.TileContext(nc) as tc:
        _body(nc, tc, locals())
    return nc


def _body(nc, tc, t):
    xT, wqT, wkT, wvT, woT = t["xT"], t["wqT"], t["wkT"], t["wvT"], t["woT"]
    bq, bk, bv, bo, padb, tri = t["bq"], t["bk"], t["bv"], t["bo"], t["padb"], t["tri"]
    out = t["out"]

    ctx_pools = []

    def pool(name, bufs, space="SBUF"):
        p = tc.tile_pool(name=name, bufs=bufs, space=space)
        ctx_pools.append(p)
        return p.__enter__()

    dram_pool = pool("dram", 1, space="DRAM")
    ag_in = dram_pool.tile([NQT, HC, QT], bf16)
    ag_out = dram_pool.tile([NQT, GROUP * HC, QT], bf16)

    const = pool("const", 1)
    probs_pool = pool("probs", 4)
    stage_pool = pool("stage", 3)
    agsb_pool = pool("agsb", 2)
    outsb_pool = pool("outsb", 2)

    ps_proj = pool("ps_proj", 2, space="PSUM")
    ps_st = pool("ps_st", 2, space="PSUM")
    ps_ot = pool("ps_ot", 2, space="PSUM")
    qk_ctx = tc.tile_pool(name="qk", bufs=1)
    qk_pool = qk_ctx.__enter__()
    ctx_pools.append(qk_ctx)
    vh_ctx = tc.tile_pool(name="vh", bufs=1)
    vh_pool = vh_ctx.__enter__()
    ctx_pools.append(vh_ctx)

    # ---- small consts first (tiny DMAs) ---------------------------------
    bq_t = const.tile([128, 2], f32)
    nc.sync.dma_start(bq_t[:], bq[:])
    bk_t = const.tile([128, 2], f32)
    nc.sync.dma_start(bk_t[:], bk[:])
    bo_t = const.tile([128, 2], f32)
    nc.sync.dma_start(bo_t[:], bo[:])
    padb_t = const.tile([128, NKT], f32)
    nc.sync.dma_start(padb_t[:], padb[:])
    tri_t = const.tile([128, 128], f32)
    nc.sync.dma_start(tri_t[:], tri[:])
    bv_row = const.tile([1, HC], bf16)
    nc.sync.dma_start(bv_row[:], bv[:])

    # ---- resident inputs, ordered so tile-0 work is ready first ----------
    xt = const.tile([128, KCH, S], bf16)      # xT, chunk-major
    wq_t = const.tile([128, KCH, HC], bf16)
    wk_t = const.tile([128, KCH, HC], bf16)
    wv_t = const.tile([128, KCH, HC], bf16)
    wo_t = const.tile([128, KCH, HC], bf16)

    xT_r = xT.rearrange("(c p) s -> p c s", p=128)
    wqT_r = wqT.rearrange("(c p) j -> p c j", p=128)
    wkT_r = wkT.rearrange("(c p) j -> p c j", p=128)
    wvT_r = wvT.rearrange("(c p) j -> p c j", p=128)
    woT_r = woT.rearrange("(c p) j -> p c j", p=128)
    for k in range(KCH):
        nc.sync.dma_start(wk_t[:, k], wkT_r[:, k])
        nc.sync.dma_start(wq_t[:, k], wqT_r[:, k])
        nc.sync.dma_start(xt[:, k, 0:QT], xT_r[:, k, 0:QT])
    for k in range(KCH):
        nc.sync.dma_start(wv_t[:, k], wvT_r[:, k])
    for c in range(1, NQT):
        for k in range(KCH):
            nc.sync.dma_start(
                xt[:, k, c * QT : (c + 1) * QT], xT_r[:, k, c * QT : (c + 1) * QT]
            )
    for k in range(KCH):
        nc.sync.dma_start(wo_t[:, k], woT_r[:, k])

    ones_f = const.tile([128, 128], f32)
    nc.any.memset(ones_f[:], 1.0)
    ones_b = const.tile([1, 128], bf16)
    nc.vector.tensor_copy(ones_b[0:1, :], ones_f[0:1, :])
    bvb = const.tile([128, HC], f32)
    bv_ps = ps_proj.tile([128, HC], f32, tag="proj")
    nc.tensor.matmul(bv_ps[:], ones_b[:], bv_row[:], start=True, stop=True)
    nc.vector.tensor_copy(bvb[:], bv_ps[:])

    # projection outputs
    qh_t = qk_pool.tile([128, 2, S], bf16)    # qhT: [j-in-tile, j-tile, s]
    kh_t = qk_pool.tile([128, 2, S], bf16)
    vh_t = vh_pool.tile([128, NKT, HPC, HD + 1], bf16)  # [s%128, s//128, head, d'+ones]
    nc.vector.tensor_copy(
        vh_t[:, :, :, HD : HD + 1].rearrange("p t h o -> p (t h) o"),
        ones_f[:, 0 : NKT * HPC].rearrange("p (f o) -> p f o", o=1),
    )

    def proj_qk(w_t, b_t, out_t, jt, c):
        """one [128, 512] tile of qhT/khT: out partition=j, free=s."""
        ps = ps_proj.tile([128, QT], f32, tag="proj")
        for k in range(KCH):
            nc.tensor.matmul(
                ps[:],
                w_t[:, k, jt * 128 : (jt + 1) * 128],
                xt[:, k, c * QT : (c + 1) * QT],
                start=(k == 0),
                stop=(k == KCH - 1),
            )
            if k % 2 == 1:
                yield
        with nc.allow_low_precision(reason="bf16 activations"):
            nc.vector.tensor_scalar_add(
                out_t[:, jt, c * QT : (c + 1) * QT], ps[:], b_t[:, jt : jt + 1]
            )

    def proj_v(st_):
        """one s-tile of vh: out partition=s, free=[4 heads x 64]."""
        ps = ps_proj.tile([128, HC], f32, tag="proj")
        for k in range(KCH):
            nc.tensor.matmul(
                ps[:],
                xt[:, k, st_ * 128 : (st_ + 1) * 128],
                wv_t[:, k, :],
                start=(k == 0),
                stop=(k == KCH - 1),
            )
            if k % 2 == 1:
                yield
        with nc.allow_low_precision(reason="bf16 activations"):
            nc.vector.tensor_tensor(
                vh_t[:, st_, :, 0:HD],
                ps[:].rearrange("p (h d) -> p h d", h=HPC),
                bvb[:].rearrange("p (h d) -> p h d", h=HPC),
                mybir.AluOpType.add,
            )

    def attention_qtile(qi, filler=None):
        q0 = qi * QT
        nk = 4 * (qi + 1)
        for pair in range(2):  # heads (2p, 2p+1) at partitions 0-63 / 64-127, jt=pair
            ot0 = ps_ot.tile([HD + 1, QT], f32, tag="ot")
            ot1 = ps_ot.tile([HD + 1, QT], f32, tag="ot")
            ots = (ot0, ot1)
            for kt in range(nk):
                if filler is not None:
                    filler()
                k0 = kt * 128
                d0 = max(0, k0 - q0)  # first valid q-col in this tile
                st = ps_st.tile([128, 2, QT], f32, tag="st")
                diag = k0 >= q0
                for hh in range(2):
                    nc.tensor.matmul(
                        st[:, hh, d0:QT],
                        kh_t[hh * 64 : hh * 64 + 64, pair, k0 : k0 + 128],
                        qh_t[hh * 64 : hh * 64 + 64, pair, q0 + d0 : q0 + QT],
                        start=True,
                        stop=True,
                    )
                if diag:  # causal mask on the [128,128] diagonal block (DVE)
                    nc.vector.tensor_tensor(
                        st[:, :, d0 : d0 + 128],
                        st[:, :, d0 : d0 + 128],
                        tri_t[:].rearrange(
                            "p (o n) -> p o n", o=1
                        ).broadcast_to([128, 2, 128]),
                        mybir.AluOpType.add,
                    )
                probs = probs_pool.tile([128, 2, QT], bf16, tag="probs")
                with nc.allow_low_precision(reason="bf16 probs"):
                    nc.scalar.activation(
                        probs[:, :, d0:QT],
                        st[:, :, d0:QT],
                        mybir.ActivationFunctionType.Exp,
                        bias=padb_t[:, kt : kt + 1],
                        scale=float(SCALE),
                    )
                for hh in range(2):
                    h = 2 * pair + hh
                    nc.tensor.matmul(
                        ots[hh][:, d0:QT],
                        vh_t[:, kt, h, :],
                        probs[:, hh, d0:QT],
                        start=(kt == 0),
                        stop=(kt == nk - 1),
                    )
            # normalize by the softmax denominator (row HD) and stage bf16
            if filler is not None:
                filler()
            den_r = stage_pool.tile([1, 2, QT], bf16, tag="den")
            with nc.allow_low_precision(reason="bf16 denominators"):
                nc.vector.reciprocal(den_r[0:1, 0, :], ot0[HD : HD + 1, :])
                nc.vector.reciprocal(den_r[0:1, 1, :], ot1[HD : HD + 1, :])
            bcst = ps_st.tile([HD, 2, QT], f32, tag="st")
            for hh in range(2):
                nc.tensor.matmul(
                    bcst[:, hh, :], ones_b[0:1, 0:HD], den_r[0:1, hh, :],
                    start=True, stop=True,
                )
            # DVE can read only one non-scalar PSUM input; bounce the
            # broadcast denominators through SBUF on the scalar engine.
            bcst_sb = stage_pool.tile([HD, 2, QT], f32, tag="bcst")
            nc.scalar.copy(bcst_sb[:], bcst[:])
            for hh in range(2):
                if filler is not None:
                    filler()
                h = 2 * pair + hh
                stg = stage_pool.tile([HD, QT], bf16, tag="stg")
                with nc.allow_low_precision(reason="bf16 staging"):
                    nc.vector.tensor_tensor(
                        stg[:], ots[hh][0:HD, :], bcst_sb[:, hh, :],
                        mybir.AluOpType.mult,
                    )
                nc.sync.dma_start(
                    ag_in[qi, h * HD : (h + 1) * HD, :], stg[:]
                )

    def oproj_tile(c, ag_sb):
        """O-projection columns [256] for q-tile c from gathered heads."""
        for jh in range(2):
            ps = ps_proj.tile([128, QT], f32, tag="proj")
            for dc in range(KCH):
                nc.tensor.matmul(
                    ps[:],
                    wo_t[:, dc, jh * 128 : (jh + 1) * 128],
                    ag_sb[:, dc, :],
                    start=(dc == 0),
                    stop=(dc == KCH - 1),
                )
                if dc % 2 == 1:
                    yield
            osb = outsb_pool.tile([128, QT], f32, tag="osb")
            nc.vector.tensor_scalar_add(osb[:], ps[:], bo_t[:, jh : jh + 1])
            nc.sync.dma_start(out[c, jh], osb[:])

    # ---- emission: projections + O-proj finely interleaved with attention -
    def proj_units(c):
        units = []
        for jt in range(2):
            units.append(lambda jt=jt, c=c: proj_qk(wk_t, bk_t, kh_t, jt, c))
            units.append(lambda jt=jt, c=c: proj_qk(wq_t, bq_t, qh_t, jt, c))
        for st_ in range(4 * c, 4 * c + 4):
            units.append(lambda st_=st_: proj_v(st_))
        return units

    class Filler:
        def __init__(self, units, budget):
            self.units = list(units)
            self.gen = None
            self.budget = budget

        def __call__(self):
            for _ in range(self.budget):
                if self.gen is None:
                    if not self.units:
                        return
                    self.gen = self.units.pop(0)()
                try:
                    next(self.gen)
                except StopIteration:
                    self.gen = None

        def flush(self):
            while self.units or self.gen is not None:
                if self.gen is None:
                    self.gen = self.units.pop(0)()
                for _ in self.gen:
                    pass
                self.gen = None

    Filler(proj_units(0), 1).flush()
    ag_sbs = {}
    for c in range(NQT):
        pending = proj_units(c + 1) if c + 1 < NQT else []
        if c - 2 >= 0:
            pending.append(lambda c2=c - 2: oproj_tile(c2, ag_sbs[c2]))
        n_att = 2 * 4 * (c + 1) + 6
        total_steps = len(pending) * 5
        budget = max(1, (total_steps + n_att - 1) // n_att)
        filler = Filler(pending, budget)
        attention_qtile(c, filler)
        filler.flush()
        # AllGather this q-tile's normalized head outputs across the group
        nc.gpsimd.collective_compute(
            "AllGather",
            mybir.AluOpType.bypass,
            replica_groups=REPLICA_GROUPS,
            ins=[ag_in[c]],
            outs=[ag_out[c]],
        )
        # prefetch the gathered tile for the overlapped O-projection
        ag_sb = agsb_pool.tile([128, KCH, QT], bf16, tag="agsb")
        nc.gpsimd.dma_start(
            ag_sb[:], ag_out[c].rearrange("(k p) q -> p k q", p=128)
        )
        ag_sbs[c] = ag_sb

    for c in (NQT - 2, NQT - 1):
        for _ in oproj_tile(c, ag_sbs[c]):
            pass

    for p in reversed(ctx_pools):
        p.__exit__(None, None, None)


# ---- host-side marshalling ----------------------------------------------


def _bf16(a):
    import ml_dtypes

    return np.asarray(a, dtype=np.float32).astype(ml_dtypes.bfloat16)


def make_inputs(q, pad_mask, Wq, bq, Wk, bk, Wv, bv, Wo, bo):
    """Build the 8 per-core input maps from full inputs."""
    tri_m = np.where(
        np.arange(128)[:, None] > np.arange(128)[None, :], np.float32(NEG), np.float32(0)
    ).astype(np.float32)
    in_maps = []
    xTs = [_bf16(np.ascontiguousarray(q[b].T)) for b in range(B)]
    padbs = [
        np.ascontiguousarray(
            np.where(pad_mask[b], np.float32(NEG), np.float32(0))
            .astype(np.float32)
            .reshape(NKT, 128)
            .T
        )
        for b in range(B)
    ]
    WoT = np.ascontiguousarray(Wo.T)
    for core in range(8):
        b, r = divmod(core, GROUP)
        sl = slice(r * HC, (r + 1) * HC)
        in_maps.append(
            {
                "xT": xTs[b],
                "wqT": _bf16(np.ascontiguousarray(Wq[sl, :].T)),
                "wkT": _bf16(np.ascontiguousarray(Wk[sl, :].T)),
                "wvT": _bf16(np.ascontiguousarray(Wv[sl, :].T)),
                "woT": _bf16(np.ascontiguousarray(WoT[:, sl])),
                "bq": np.ascontiguousarray(np.asarray(bq)[sl].reshape(2, 128).T),
                "bk": np.ascontiguousarray(np.asarray(bk)[sl].reshape(2, 128).T),
                "bv": _bf16(np.asarray(bv)[sl].reshape(1, HC)),
                "bo": np.ascontiguousarray(np.asarray(bo)[sl].reshape(2, 128).T),
                "padb": padbs[b],
                "tri": tri_m,
            }
        )
    return in_maps


def assemble_output(results):
    full = np.empty((B, S, D), dtype=np.float32)
    for core in range(8):
        b, r = divmod(core, GROUP)
        o = results[core]["out"]  # [NQT, 2, 128, QT]
        # out[c, jh, j, q] = output row c*512+q, col r*256 + jh*128 + j
        full[b, :, r * HC : (r + 1) * HC] = (
            o.transpose(0, 3, 1, 2).reshape(S, HC)
        )
    return full


_NC_CACHE = [None]


def kernel(**inputs):
    """Full-input MHA forward. inputs: q, pad_mask, Wq, bq, Wk, bk, Wv, bv,
    Wo, bo (as produced by setup_inputs). Returns [B, S, D] float32."""
    if _NC_CACHE[0] is None:
        _NC_CACHE[0] = build()
    nc = _NC_CACHE[0]
    inputs = {k: np.asarray(v) for k, v in inputs.items()}
    in_maps = make_inputs(**inputs)
    res = run_bass_kernel_spmd(nc, in_maps, list(range(8)))
    return assemble_output(res.results)
